# revision 3
# baseline (speedup 1.0000x reference)
"""Trainium2 Bass kernel for nn_DecoderPolicyGradient (teacher-forced LSTM decoder).

Model: B=128, T=20, E=H=512, V=10000.
  xs[t] = features (t=0) | embed(captions[:, t-1])
  (h, c) = LSTM(xs[t], (h, c));  logits[t] = h @ W_lin.T + b_lin
  out = logits, time-major flattened: [T*B, V] fp32.

Sharding: pure data-parallel over batch, B/8 = 16 rows per NeuronCore, no
collectives. Per-core plan (everything "transposed" so the 128-partition
axis carries hidden/gate dims and batch lives in the free dim):

  1. XgT[2048, 320] = W_ih @ xs.T + (b_ih + b_hh): one batched matmul over
     all 20 steps (lhsT = W_ihT tiles, rhs = xsT); bias added by ACT during
     the psum->SBUF copy as a per-partition bias.
  2. 20 serial LSTM steps at B=16: gatesT[2048, 16] = W_hh @ h + XgT[:, t]
     as 16 m-tiles of [128, 16] (lhsT = W_hhT tiles [128, 128] bf16 -> FWL;
     measured ~37 ns/LDW+MM pair). The cell runs in [128, 4, 16] layout
     (full 128 ACT/DVE lanes); h_t is produced directly in the transposed
     layout that the next step's matmul and the logits lhsT need.
  3. logits[320, 10000] = H @ W_lin.T + b_lin: 3 m-chunks x 20 n-slices,
     b_lin folded in via a K=1 matmul with a ones-vector lhsT.

Host side does data movement only: embedding row gather, weight re-layouts,
and the final 8 x [320, 10000] -> [2560, 10000] reassembly.
"""

import sys

sys.path.insert(0, "/opt/trn_rl_repo")

from contextlib import ExitStack

import ml_dtypes
import numpy as np

import concourse.mybir as mybir
import concourse.tile as tile
from concourse import bacc
from concourse.bass_utils import run_bass_kernel_spmd

BF16 = mybir.dt.bfloat16
F32 = mybir.dt.float32
AF = mybir.ActivationFunctionType

B, T, E, H, V = 128, 20, 512, 512, 10000
NC = 8
BL = B // NC  # 16 batch rows per core
R = BL * T  # 320 output rows per core
KT = 4  # k-tiles of 128 over E/H
GT = 16  # m-tiles of 128 over 4H
VS = 512  # vocab n-slice width
M_CHUNKS = ((0, 128), (128, 128), (256, 64))  # logits m-chunks (start, rows)

_cache = {}


def _n_slices():
    out = []
    s = 0
    while s < V:
        out.append((s, min(VS, V - s)))
        s += VS
    return out


def _build_nc():
    nc = bacc.Bacc("TRN2", target_bir_lowering=False, debug=False)

    xsT_d = nc.dram_tensor("xsT", [128, KT, R], BF16, kind="ExternalInput").ap()
    wihT_d = nc.dram_tensor("wihT", [128, KT, 4 * H], BF16, kind="ExternalInput").ap()
    whhT_d = nc.dram_tensor("whhT", [128, KT, 4 * H], BF16, kind="ExternalInput").ap()
    bsum_d = nc.dram_tensor("bsum", [128, GT], F32, kind="ExternalInput").ap()
    wlinT_d = nc.dram_tensor("wlinT", [128, KT, V], BF16, kind="ExternalInput").ap()
    blin_d = nc.dram_tensor("blin", [1, V], BF16, kind="ExternalInput").ap()
    h0T_d = nc.dram_tensor("h0T", [128, KT, BL], BF16, kind="ExternalInput").ap()
    c0T_d = nc.dram_tensor("c0T", [128, KT, BL], F32, kind="ExternalInput").ap()
    out_d = nc.dram_tensor("out", [R, V], F32, kind="ExternalOutput").ap()

    with tile.TileContext(nc) as tc, ExitStack() as ctx:
        const = ctx.enter_context(tc.tile_pool(name="const", bufs=1))
        work = ctx.enter_context(tc.tile_pool(name="work", bufs=2))
        stage = ctx.enter_context(tc.tile_pool(name="stage", bufs=6))
        psum = ctx.enter_context(tc.tile_pool(name="psum", bufs=2, space="PSUM"))

        # ---- persistent SBUF tensors + input DMAs
        xsT = const.tile([128, KT, R], BF16)
        wihT = const.tile([128, KT, 4 * H], BF16)
        bsum = const.tile([128, GT], F32)
        whhT = const.tile([128, KT, 4 * H], BF16)
        h0T = const.tile([128, KT, BL], BF16)
        c0T = const.tile([128, KT, BL], F32)
        blin = const.tile([1, V], BF16)
        ones = const.tile([1, 128], BF16)
        wlinT = const.tile([128, KT, V], BF16)
        xgT = const.tile([128, GT, R], BF16)
        hstore = const.tile([128, KT, R], BF16)

        nc.sync.dma_start(xsT[:], xsT_d[:])
        nc.sync.dma_start(wihT[:], wihT_d[:])
        nc.sync.dma_start(bsum[:], bsum_d[:])
        nc.sync.dma_start(whhT[:], whhT_d[:])
        nc.sync.dma_start(h0T[:], h0T_d[:])
        nc.sync.dma_start(c0T[:], c0T_d[:])
        nc.sync.dma_start(blin[:], blin_d[:])
        nc.gpsimd.memset(ones[:], 1.0)
        # big W_lin load: split across queues; consumed only in phase 3
        for q in range(8):
            s = q * (V // 8)
            nc.sync.dma_start(
                wlinT[:, :, s : s + V // 8], wlinT_d[:, :, s : s + V // 8]
            )

        # ---- phase 1: XgT[2048, R] = W_ih @ xs.T + bsum
        for m in range(GT):
            pxg = psum.tile([128, R], F32, tag=f"pg{m % 4}")
            for k in range(KT):
                nc.tensor.matmul(
                    pxg[:],
                    wihT[:, k, m * 128 : (m + 1) * 128],
                    xsT[:, k, :],
                    start=(k == 0),
                    stop=(k == KT - 1),
                )
            nc.scalar.activation(
                xgT[:, m, :], pxg[:], AF.Identity, bias=bsum[:, m : m + 1]
            )

        # ---- phase 2: 20 serial LSTM steps (B = 16)
        cT_prev = c0T
        hT_prev = h0T  # AP views [128, KT, BL]
        for t in range(T):
            lo = t * BL
            hs = lambda k: hT_prev[:, k, :]
            # gate matmuls: 4 psum tiles (i, f, g, o), each [128, 4, 16]
            pg = []
            for g in range(4):
                p = psum.tile([128, 4, BL], F32, tag=f"pg{g}")
                pg.append(p)
                for mi in range(4):
                    m = g * 4 + mi
                    for k in range(KT):
                        nc.tensor.matmul(
                            p[:, mi, :],
                            whhT[:, k, m * 128 : (m + 1) * 128],
                            hs(k),
                            start=(k == 0),
                            stop=(k == KT - 1),
                        )
            # add XgT slice (psum f32 + sbuf bf16 -> sbuf bf16), then acts
            acts = []
            for g in range(4):
                gate = work.tile([128, 4, BL], BF16, tag=f"gate{g}")
                nc.vector.tensor_add(
                    gate[:], pg[g][:], xgT[:, 4 * g : 4 * g + 4, lo : lo + BL]
                )
                a = work.tile([128, 4, BL], BF16, tag=f"act{g}")
                fn = AF.Tanh if g == 2 else AF.Sigmoid
                nc.scalar.activation(a[:], gate[:], fn)
                acts.append(a)
            i_a, f_a, g_a, o_a = acts
            ig = work.tile([128, 4, BL], F32, tag="ig")
            fc = work.tile([128, 4, BL], F32, tag="fc")
            nc.vector.tensor_mul(ig[:], i_a[:], g_a[:])
            nc.vector.tensor_mul(fc[:], f_a[:], cT_prev[:])
            c_new = work.tile([128, 4, BL], F32, tag="c")
            nc.vector.tensor_add(c_new[:], fc[:], ig[:])
            tc_b = work.tile([128, 4, BL], BF16, tag="tanhc")
            nc.scalar.activation(tc_b[:], c_new[:], AF.Tanh)
            h_new = hstore[:, :, lo : lo + BL]
            nc.vector.tensor_mul(h_new, o_a[:], tc_b[:])
            cT_prev = c_new
            hT_prev = h_new

        # ---- phase 3: logits[320, 10000] = H @ W_lin.T + b_lin
        for ci, (ms, rows) in enumerate(M_CHUNKS):
            for ni, (s, w) in enumerate(_n_slices()):
                pl = psum.tile([128, VS], F32, tag=f"pg{ni % 4}")
                for k in range(KT):
                    nc.tensor.matmul(
                        pl[:rows, :w],
                        hstore[:, k, ms : ms + rows],
                        wlinT[:, k, s : s + w],
                        start=(k == 0),
                        stop=False,
                    )
                nc.tensor.matmul(
                    pl[:rows, :w],
                    ones[:, :rows],
                    blin[:, s : s + w],
                    start=False,
                    stop=True,
                )
                ot = stage.tile([128, VS], F32, tag=f"o{ni % 3}")
                if ni % 2 == 0:
                    nc.vector.tensor_copy(ot[:rows, :w], pl[:rows, :w])
                else:
                    nc.scalar.copy(ot[:rows, :w], pl[:rows, :w])
                nc.sync.dma_start(out_d[ms : ms + rows, s : s + w], ot[:rows, :w])

    nc.compile()
    return nc


def _prep_inputs(features, captions, h0, c0, embed_w, W_ih, W_hh, b_ih, b_hh,
                 W_lin, b_lin):
    """Host-side layout prep (data movement only). Returns per-core in_maps."""
    bf = ml_dtypes.bfloat16
    f32 = np.float32

    features = np.asarray(features, f32)
    captions = np.asarray(captions)
    h0 = np.asarray(h0, f32)
    c0 = np.asarray(c0, f32)
    embed_w = np.asarray(embed_w, f32)
    W_ih = np.asarray(W_ih, f32)
    W_hh = np.asarray(W_hh, f32)
    b_ih = np.asarray(b_ih, f32)
    b_hh = np.asarray(b_hh, f32)
    W_lin = np.asarray(W_lin, f32)
    b_lin = np.asarray(b_lin, f32)

    # xs: [B, T, E] = [features, embed(captions[:, :T-1])]
    xs = np.empty((B, T, E), f32)
    xs[:, 0, :] = features
    xs[:, 1:, :] = embed_w[captions[:, : T - 1]]

    def to_kpm(w):  # [X, 4H] with X = E: -> [128, KT, 4H] (p, k, m)
        return np.ascontiguousarray(
            w.reshape(KT, 128, w.shape[1]).transpose(1, 0, 2)
        )

    wihT = to_kpm(W_ih.T).astype(bf)  # W_ih.T: [E, 4H]
    whhT = to_kpm(W_hh.T).astype(bf)
    wlinT = to_kpm(W_lin.T).astype(bf)  # [E=512, V] -> [128, 4, V]
    bsum = np.ascontiguousarray((b_ih + b_hh).reshape(GT, 128).T).astype(f32)
    blin = b_lin.reshape(1, V).astype(bf)

    in_maps = []
    for j in range(NC):
        sl = slice(j * BL, (j + 1) * BL)
        # xsT: [128, KT, R] with R = (t, b) t-major; xsT[p,k,t*BL+b] = xs[b,t,k*128+p]
        x = xs[sl]  # [BL, T, E]
        xsT = x.transpose(2, 1, 0).reshape(KT, 128, T * BL).transpose(1, 0, 2)
        h0T = h0[sl].T.reshape(KT, 128, BL).transpose(1, 0, 2)
        c0T = c0[sl].T.reshape(KT, 128, BL).transpose(1, 0, 2)
        in_maps.append(
            {
                "xsT": np.ascontiguousarray(xsT).astype(bf),
                "wihT": wihT,
                "whhT": whhT,
                "bsum": bsum,
                "wlinT": wlinT,
                "blin": blin,
                "h0T": np.ascontiguousarray(h0T).astype(bf),
                "c0T": np.ascontiguousarray(c0T).astype(f32),
            }
        )
    return in_maps


def kernel(**inputs) -> np.ndarray:
    maxlen = int(inputs.get("maxlen", T))
    assert maxlen == T, f"kernel hardcodes T={T}, got maxlen={maxlen}"
    if "nc" not in _cache:
        _cache["nc"] = _build_nc()
    nc = _cache["nc"]
    in_maps = _prep_inputs(
        inputs["features"], inputs["captions"], inputs["h0"], inputs["c0"],
        inputs["embed_w"], inputs["W_ih"], inputs["W_hh"], inputs["b_ih"],
        inputs["b_hh"], inputs["W_lin"], inputs["b_lin"],
    )
    res = run_bass_kernel_spmd(nc, in_maps, list(range(NC)))
    # reassemble: core j rows (t*BL + b) -> full rows (t*B + j*BL + b)
    out = np.empty((T * B, V), np.float32)
    ov = out.reshape(T, NC, BL, V)
    for j in range(NC):
        ov[:, j] = res.results[j]["out"].reshape(T, BL, V)
    return out


# revision 4
# speedup vs baseline: 1.2749x; 1.2749x over previous
"""Trainium2 Bass kernel for nn_DecoderPolicyGradient (teacher-forced LSTM decoder).

Model: B=128, T=20, E=H=512, V=10000.
  xs[t] = features (t=0) | embed(captions[:, t-1])
  (h, c) = LSTM(xs[t], (h, c));  logits[t] = h @ W_lin.T + b_lin
  out = logits, time-major flattened: [T*B, V] fp32.

Sharding: pure data-parallel over batch, B/8 = 16 rows per NeuronCore, no
collectives. Per-core plan (everything "transposed": the 128-partition axis
carries hidden/gate dims, batch lives in the free dim):

  1. XgT[2048, 320] = W_ih @ xs.T + (b_ih + b_hh): one batched matmul over
     all 20 steps (lhsT = W_ihT tiles, rhs = xsT); the bias rides the ACT
     psum->SBUF copy as a per-partition bias.
  2. 20 serial LSTM steps at B=16: gatesT[2048, 16] = W_hh @ h + XgT[:, t]
     as 16 m-tiles of [128, 16] (lhsT = W_hhT [128, 128] bf16 -> FWL,
     ~37 ns/LDW+MM pair warm). The cell runs in [128, 4, 16] layout (full
     128 ACT/DVE lanes); h_t is produced directly in the transposed layout
     the next matmul and the logits lhsT need - no transposes anywhere.
  3. logits[320, 10000] = H @ W_lin.T (+ b_lin via a K=1 ones matmul, only
     when b_lin != 0): 3 m-chunks x 20 n-slices. Chunk ci's matmuls are
     interleaved into the PE tail-gaps of recurrence steps > 8*(ci+1)-1,
     which both absorbs the logits phase and keeps the PE HAM clock warm.

Host side does data movement only: embedding row gather, weight re-layouts,
and the final 8 x [320, 10000] -> [2560, 10000] reassembly.
"""

import sys

sys.path.insert(0, "/opt/trn_rl_repo")

from contextlib import ExitStack

import ml_dtypes
import numpy as np

import concourse.mybir as mybir
import concourse.tile as tile
from concourse import bacc
from concourse.bass_utils import run_bass_kernel_spmd

BF16 = mybir.dt.bfloat16
F32 = mybir.dt.float32
AF = mybir.ActivationFunctionType

B, T, E, H, V = 128, 20, 512, 512, 10000
NC = 8
BL = B // NC  # 16 batch rows per core
R = BL * T  # 320 output rows per core
KT = 4  # k-tiles of 128 over E/H
GT = 16  # m-tiles of 128 over 4H
VS = 512  # vocab n-slice width
M_CHUNKS = ((0, 128), (128, 128), (256, 64))  # logits m-chunks (start, rows)
N_SLICES = [(s, min(VS, V - s)) for s in range(0, V, VS)]

_cache = {}


def _logits_schedule():
    """Map step t -> list of logits units (ci, ni) emitted after that step.
    Chunk ci is ready once step 8*(ci+1)-1 is done (chunk 2: after loop)."""
    per_step = {t: [] for t in range(T)}
    post = []
    units = [(ci, ni) for ci in range(3) for ni in range(len(N_SLICES))]
    avail = {0: 8, 1: 16, 2: T}  # first step index whose tail can host chunk
    q = []
    ui = 0
    for t in range(8, T):
        # refill queue with units that became available
        while ui < len(units) and avail[units[ui][0]] <= t:
            q.append(units[ui])
            ui += 1
        take, q = q[:2], q[2:]
        per_step[t] = take
    post = q + units[ui:]
    return per_step, post


def _build_nc(use_blin):
    nc = bacc.Bacc("TRN2", target_bir_lowering=False, debug=False)

    xsT_d = nc.dram_tensor("xsT", [128, KT, R], BF16, kind="ExternalInput").ap()
    wihT_d = nc.dram_tensor("wihT", [128, KT, 4 * H], BF16, kind="ExternalInput").ap()
    whhT_d = nc.dram_tensor("whhT", [128, KT, 4 * H], BF16, kind="ExternalInput").ap()
    bsum_d = nc.dram_tensor("bsum", [128, GT], F32, kind="ExternalInput").ap()
    wlinT_d = nc.dram_tensor("wlinT", [128, KT, V], BF16, kind="ExternalInput").ap()
    blin_d = nc.dram_tensor("blin", [1, V], BF16, kind="ExternalInput").ap()
    h0T_d = nc.dram_tensor("h0T", [128, KT, BL], BF16, kind="ExternalInput").ap()
    c0T_d = nc.dram_tensor("c0T", [128, KT, BL], F32, kind="ExternalInput").ap()
    out_d = nc.dram_tensor("out", [R, V], F32, kind="ExternalOutput").ap()

    with tile.TileContext(nc) as tc, ExitStack() as ctx:
        const = ctx.enter_context(tc.tile_pool(name="const", bufs=1))
        work = ctx.enter_context(tc.tile_pool(name="work", bufs=2))
        stage = ctx.enter_context(tc.tile_pool(name="stage", bufs=6))
        psum = ctx.enter_context(tc.tile_pool(name="psum", bufs=1, space="PSUM"))

        # ---- persistent SBUF tensors + input DMAs
        xsT = const.tile([128, KT, R], BF16)
        wihT = const.tile([128, KT, 4 * H], BF16)
        bsum = const.tile([128, GT], F32)
        whhT = const.tile([128, KT, 4 * H], BF16)
        h0T = const.tile([128, KT, BL], BF16)
        c0T = const.tile([128, KT, BL], F32)
        blin = const.tile([1, V], BF16)
        ones = const.tile([1, 128], BF16)
        wlinT = const.tile([128, KT, V], BF16)
        xgT = const.tile([128, GT, R], BF16)
        hstore = const.tile([128, KT, R], BF16)

        nc.sync.dma_start(xsT[:], xsT_d[:])
        nc.sync.dma_start(wihT[:, :, 0 : 4 * 128], wihT_d[:, :, 0 : 4 * 128])
        nc.sync.dma_start(bsum[:], bsum_d[:])
        nc.sync.dma_start(wihT[:, :, 4 * 128 :], wihT_d[:, :, 4 * 128 :])
        nc.sync.dma_start(whhT[:], whhT_d[:])
        nc.sync.dma_start(h0T[:], h0T_d[:])
        nc.sync.dma_start(c0T[:], c0T_d[:])
        if use_blin:
            nc.sync.dma_start(blin[:], blin_d[:])
        nc.gpsimd.memset(ones[:], 1.0)
        # big W_lin load: split across queues; consumed only by logits units
        for q in range(8):
            s = q * (V // 8)
            nc.sync.dma_start(
                wlinT[:, :, s : s + V // 8], wlinT_d[:, :, s : s + V // 8]
            )

        # ---- phase 1: XgT[2048, R] = W_ih @ xs.T + bsum
        for m in range(GT):
            pxg = psum.tile([128, R], F32, tag=f"pl{m % 2}")
            for k in range(KT):
                nc.tensor.matmul(
                    pxg[:],
                    wihT[:, k, m * 128 : (m + 1) * 128],
                    xsT[:, k, :],
                    start=(k == 0),
                    stop=(k == KT - 1),
                )
            nc.scalar.activation(
                xgT[:, m, :], pxg[:], AF.Identity, bias=bsum[:, m : m + 1]
            )

        # ---- logits unit emitter (phase 3, interleaved into phase 2)
        def emit_logits_unit(ci, ni):
            ms, rows = M_CHUNKS[ci]
            s, w = N_SLICES[ni]
            pl = psum.tile([128, VS], F32, tag=f"pl{ni % 2}")
            for k in range(KT):
                nc.tensor.matmul(
                    pl[:rows, :w],
                    hstore[:, k, ms : ms + rows],
                    wlinT[:, k, s : s + w],
                    start=(k == 0),
                    stop=(k == KT - 1) and not use_blin,
                )
            if use_blin:
                nc.tensor.matmul(
                    pl[:rows, :w],
                    ones[:, :rows],
                    blin[:, s : s + w],
                    start=False,
                    stop=True,
                )
            ot = stage.tile([128, VS], F32, tag=f"o{ni % 3}")
            if ni % 3 == 2:
                nc.scalar.copy(ot[:rows, :w], pl[:rows, :w])
            else:
                nc.vector.tensor_copy(ot[:rows, :w], pl[:rows, :w])
            nc.sync.dma_start(out_d[ms : ms + rows, s : s + w], ot[:rows, :w])

        sched, post = _logits_schedule()

        # ---- phase 2: 20 serial LSTM steps (B = 16)
        cT_prev = c0T
        hT_prev = h0T
        for t in range(T):
            lo = t * BL
            pg = []
            for g in range(4):
                p = psum.tile([128, 4, BL], F32, tag=f"pg{g}")
                pg.append(p)
                for mi in range(4):
                    m = g * 4 + mi
                    for k in range(KT):
                        nc.tensor.matmul(
                            p[:, mi, :],
                            whhT[:, k, m * 128 : (m + 1) * 128],
                            hT_prev[:, k, :],
                            start=(k == 0),
                            stop=(k == KT - 1),
                        )
            gates = work.tile([128, GT, BL], BF16, tag="gates")
            for g in range(4):
                nc.vector.tensor_add(
                    gates[:, 4 * g : 4 * g + 4, :],
                    pg[g][:],
                    xgT[:, 4 * g : 4 * g + 4, lo : lo + BL],
                )
            act_if = work.tile([128, 8, BL], BF16, tag="actif")
            act_g = work.tile([128, 4, BL], BF16, tag="actg")
            act_o = work.tile([128, 4, BL], BF16, tag="acto")
            nc.scalar.activation(act_if[:], gates[:, 0:8, :], AF.Sigmoid)
            nc.scalar.activation(act_g[:], gates[:, 8:12, :], AF.Tanh)
            nc.scalar.activation(act_o[:], gates[:, 12:16, :], AF.Sigmoid)
            ig = work.tile([128, 4, BL], F32, tag="ig")
            fc = work.tile([128, 4, BL], F32, tag="fc")
            nc.vector.tensor_mul(ig[:], act_if[:, 0:4, :], act_g[:])
            nc.vector.tensor_mul(fc[:], act_if[:, 4:8, :], cT_prev[:])
            c_new = work.tile([128, 4, BL], F32, tag="c")
            nc.vector.tensor_add(c_new[:], fc[:], ig[:])
            tc_b = work.tile([128, 4, BL], BF16, tag="tanhc")
            nc.scalar.activation(tc_b[:], c_new[:], AF.Tanh)
            h_new = hstore[:, :, lo : lo + BL]
            nc.vector.tensor_mul(h_new, act_o[:], tc_b[:])
            cT_prev = c_new
            hT_prev = h_new
            for ci, ni in sched[t]:
                emit_logits_unit(ci, ni)

        for ci, ni in post:
            emit_logits_unit(ci, ni)

    nc.compile()
    return nc


def _prep_inputs(features, captions, h0, c0, embed_w, W_ih, W_hh, b_ih, b_hh,
                 W_lin, b_lin):
    """Host-side layout prep (data movement only). Returns per-core in_maps."""
    bf = ml_dtypes.bfloat16
    f32 = np.float32

    features = np.asarray(features, f32)
    captions = np.asarray(captions)
    h0 = np.asarray(h0, f32)
    c0 = np.asarray(c0, f32)
    embed_w = np.asarray(embed_w, f32)
    W_ih = np.asarray(W_ih, f32)
    W_hh = np.asarray(W_hh, f32)
    b_ih = np.asarray(b_ih, f32)
    b_hh = np.asarray(b_hh, f32)
    W_lin = np.asarray(W_lin, f32)
    b_lin = np.asarray(b_lin, f32)

    # xs: [B, T, E] = [features, embed(captions[:, :T-1])]
    xs = np.empty((B, T, E), f32)
    xs[:, 0, :] = features
    xs[:, 1:, :] = embed_w[captions[:, : T - 1]]

    def to_kpm(w):  # [512, M] -> [128, KT, M] with row = k*128 + p
        return np.ascontiguousarray(w.reshape(KT, 128, w.shape[1]).transpose(1, 0, 2))

    wihT = to_kpm(W_ih.T).astype(bf)
    whhT = to_kpm(W_hh.T).astype(bf)
    wlinT = to_kpm(W_lin.T).astype(bf)
    bsum = np.ascontiguousarray((b_ih + b_hh).reshape(GT, 128).T).astype(f32)
    blin = b_lin.reshape(1, V).astype(bf)

    in_maps = []
    for j in range(NC):
        sl = slice(j * BL, (j + 1) * BL)
        x = xs[sl]  # [BL, T, E]
        xsT = x.transpose(2, 1, 0).reshape(KT, 128, T * BL).transpose(1, 0, 2)
        h0T = h0[sl].T.reshape(KT, 128, BL).transpose(1, 0, 2)
        c0T = c0[sl].T.reshape(KT, 128, BL).transpose(1, 0, 2)
        in_maps.append(
            {
                "xsT": np.ascontiguousarray(xsT).astype(bf),
                "wihT": wihT,
                "whhT": whhT,
                "bsum": bsum,
                "wlinT": wlinT,
                "blin": blin,
                "h0T": np.ascontiguousarray(h0T).astype(bf),
                "c0T": np.ascontiguousarray(c0T).astype(f32),
            }
        )
    return in_maps


def kernel(**inputs) -> np.ndarray:
    maxlen = int(inputs.get("maxlen", T))
    assert maxlen == T, f"kernel hardcodes T={T}, got maxlen={maxlen}"
    use_blin = bool(np.any(np.asarray(inputs["b_lin"])))
    key = ("nc", use_blin)
    if key not in _cache:
        _cache[key] = _build_nc(use_blin)
    nc = _cache[key]
    in_maps = _prep_inputs(
        inputs["features"], inputs["captions"], inputs["h0"], inputs["c0"],
        inputs["embed_w"], inputs["W_ih"], inputs["W_hh"], inputs["b_ih"],
        inputs["b_hh"], inputs["W_lin"], inputs["b_lin"],
    )
    res = run_bass_kernel_spmd(nc, in_maps, list(range(NC)))
    # reassemble: core j rows (t*BL + b) -> full rows (t*B + j*BL + b)
    out = np.empty((T * B, V), np.float32)
    ov = out.reshape(T, NC, BL, V)
    for j in range(NC):
        ov[:, j] = res.results[j]["out"].reshape(T, BL, V)
    return out


# revision 11
# speedup vs baseline: 1.3015x; 1.0209x over previous
"""Trainium2 Bass kernel for nn_DecoderPolicyGradient (teacher-forced LSTM decoder).

Model: B=128, T=20, E=H=512, V=10000.
  xs[t] = features (t=0) | embed(captions[:, t-1])
  (h, c) = LSTM(xs[t], (h, c));  logits[t] = h @ W_lin.T + b_lin
  out = logits, time-major flattened: [T*B, V] fp32.

Sharding: pure data-parallel over batch, B/8 = 16 rows per NeuronCore, no
collectives. Per-core plan (everything "transposed": the 128-partition axis
carries hidden/gate dims, batch lives in the free dim):

  1. XgT[2048, 320] = W_ih @ xs.T + (b_ih + b_hh): one batched matmul over
     all 20 steps (lhsT = W_ihT tiles, rhs = xsT); the bias rides the ACT
     psum->SBUF copy as a per-partition bias.
  2. 20 serial LSTM steps at B=16: gatesT[2048, 16] = W_hh @ h + XgT[:, t]
     as 16 m-tiles of [128, 16] (lhsT = W_hhT [128, 128] bf16 -> FWL,
     ~37 ns/LDW+MM pair warm). The cell runs in [128, 4, 16] layout (full
     128 ACT/DVE lanes); h_t is produced directly in the transposed layout
     the next matmul and the logits lhsT need - no transposes anywhere.
  3. logits[320, 10000] = H @ W_lin.T (+ b_lin via a K=1 ones matmul, only
     when b_lin != 0): 3 m-chunks x 20 n-slices. Chunk ci's matmuls are
     interleaved into the PE tail-gaps of recurrence steps > 8*(ci+1)-1,
     which both absorbs the logits phase and keeps the PE HAM clock warm.

Host side does data movement only: embedding row gather, weight re-layouts,
and the final 8 x [320, 10000] -> [2560, 10000] reassembly.
"""

import sys

sys.path.insert(0, "/opt/trn_rl_repo")

from contextlib import ExitStack

import ml_dtypes
import numpy as np

import concourse.mybir as mybir
import concourse.tile as tile
from concourse import bacc
from concourse.bass_utils import run_bass_kernel_spmd

BF16 = mybir.dt.bfloat16
F32 = mybir.dt.float32
AF = mybir.ActivationFunctionType

B, T, E, H, V = 128, 20, 512, 512, 10000
NC = 8
BL = B // NC  # 16 batch rows per core
R = BL * T  # 320 output rows per core
KT = 4  # k-tiles of 128 over E/H
GT = 16  # m-tiles of 128 over 4H
VS = 512  # vocab n-slice width
M_CHUNKS = ((0, 128), (128, 128), (256, 64))  # logits m-chunks (start, rows)
N_SLICES = [(s, min(VS, V - s)) for s in range(0, V, VS)]

_cache = {}


def _logits_schedule():
    """Map step t -> list of logits units (ci, ni) emitted after that step.
    Chunk ci is ready once step 8*(ci+1)-1 is done (chunk 2: after loop)."""
    per_step = {t: [] for t in range(T)}
    post = []
    units = [(0, ni) for ni in range(len(N_SLICES))]
    units += [(1, ni) for ni in range(len(N_SLICES))]
    units += [(2, ni) for ni in range(len(N_SLICES) // 2)]  # chunk-2 pairs
    avail = {0: 8, 1: 16, 2: T}  # first step index whose tail can host chunk
    q = []
    ui = 0
    for t in range(8, T):
        # refill queue with units that became available
        while ui < len(units) and avail[units[ui][0]] <= t:
            q.append(units[ui])
            ui += 1
        take, q = q[:3], q[3:]
        per_step[t] = take
    post = q + units[ui:]
    return per_step, post


def _build_nc(use_blin):
    nc = bacc.Bacc("TRN2", target_bir_lowering=False, debug=False)

    xsT_d = nc.dram_tensor("xsT", [128, KT, R], BF16, kind="ExternalInput").ap()
    wihT_d = nc.dram_tensor("wihT", [128, KT, 4 * H], BF16, kind="ExternalInput").ap()
    whhT_d = nc.dram_tensor("whhT", [128, KT, 4 * H], BF16, kind="ExternalInput").ap()
    bsum_d = nc.dram_tensor("bsum", [128, GT], F32, kind="ExternalInput").ap()
    wlinT_d = nc.dram_tensor("wlinT", [128, KT, V], BF16, kind="ExternalInput").ap()
    blin_d = nc.dram_tensor("blin", [1, V], BF16, kind="ExternalInput").ap()
    h0T_d = nc.dram_tensor("h0T", [128, KT, BL], BF16, kind="ExternalInput").ap()
    c0T_d = nc.dram_tensor("c0T", [128, KT, BL], F32, kind="ExternalInput").ap()
    out_d = nc.dram_tensor("out", [R, V], F32, kind="ExternalOutput").ap()

    with tile.TileContext(nc) as tc, ExitStack() as ctx:
        const = ctx.enter_context(tc.tile_pool(name="const", bufs=1))
        work = ctx.enter_context(tc.tile_pool(name="work", bufs=2))
        stage = ctx.enter_context(tc.tile_pool(name="stage", bufs=6))
        psum_g = ctx.enter_context(tc.tile_pool(name="psum_g", bufs=1, space="PSUM"))
        psum_l = ctx.enter_context(tc.tile_pool(name="psum_l", bufs=2, space="PSUM"))

        # ---- persistent SBUF tensors + input DMAs
        xsT = const.tile([128, KT, R], BF16)
        wihT = const.tile([128, KT, 4 * H], BF16)
        bsum = const.tile([128, GT], F32)
        whhT = const.tile([128, KT, 4 * H], BF16)
        h0T = const.tile([128, KT, BL], BF16)
        c0T = const.tile([128, KT, BL], F32)
        blin = const.tile([1, V], BF16)
        ones = const.tile([1, 128], BF16)
        wlinT = const.tile([128, KT, V], BF16)
        xgT = const.tile([128, GT, R], BF16)
        hstore = const.tile([128, KT, R], BF16)

        for k in range(KT):
            nc.sync.dma_start(xsT[:, k, :], xsT_d[:, k, :])
        nc.sync.dma_start(wihT[:, :, 0 : 4 * 128], wihT_d[:, :, 0 : 4 * 128])
        nc.sync.dma_start(bsum[:], bsum_d[:])
        nc.sync.dma_start(wihT[:, :, 4 * 128 :], wihT_d[:, :, 4 * 128 :])
        nc.sync.dma_start(whhT[:], whhT_d[:])
        nc.sync.dma_start(h0T[:], h0T_d[:])
        nc.sync.dma_start(c0T[:], c0T_d[:])
        if use_blin:
            nc.sync.dma_start(blin[:], blin_d[:])
        nc.gpsimd.memset(ones[:], 1.0)
        # big W_lin load: split across queues; consumed only by logits units
        for q in range(8):
            s = q * (V // 8)
            nc.sync.dma_start(
                wlinT[:, :, s : s + V // 8], wlinT_d[:, :, s : s + V // 8]
            )

        # ---- phase 1: XgT[2048, R] = W_ih @ xs.T + bsum
        for m in range(GT):
            pxg = psum_l.tile([128, R], F32, tag=f"pl{m % 2}")
            for k in range(KT):
                nc.tensor.matmul(
                    pxg[:],
                    wihT[:, k, m * 128 : (m + 1) * 128],
                    xsT[:, k, :],
                    start=(k == 0),
                    stop=(k == KT - 1),
                )
            nc.scalar.activation(
                xgT[:, m, :], pxg[:], AF.Identity, bias=bsum[:, m : m + 1]
            )

        # ---- logits unit emitter (phase 3, interleaved into phase 2)
        # chunks 0/1: one [128, <=512] psum + copy per n-slice.
        # chunk 2 (64 rows): two n-slices share one psum tile via the
        # base-partition-64 col-group, halving copy count.
        copy_flip = [0]

        def mm_group(pl_ap, ms, rows, s, w):
            for k in range(KT):
                nc.tensor.matmul(
                    pl_ap,
                    hstore[:, k, ms : ms + rows],
                    wlinT[:, k, s : s + w],
                    start=(k == 0),
                    stop=(k == KT - 1) and not use_blin,
                )
            if use_blin:
                nc.tensor.matmul(
                    pl_ap, ones[:, :rows], blin[:, s : s + w],
                    start=False, stop=True,
                )

        def emit_copy_dma(pl, rows_parts, dmas):
            ot = stage.tile([128, VS], F32, tag=f"o{copy_flip[0] % 3}")
            cp = nc.vector.tensor_copy if copy_flip[0] % 2 == 0 else nc.scalar.copy
            cp(ot[:rows_parts, :], pl[:rows_parts, :])
            copy_flip[0] += 1
            for (ms, rows, s, w, p0) in dmas:
                nc.sync.dma_start(
                    out_d[ms : ms + rows, s : s + w], ot[p0 : p0 + rows, :w]
                )

        def emit_logits_unit(ci, ni):
            ms, rows = M_CHUNKS[ci]
            if rows == 128:
                s, w = N_SLICES[ni]
                pl = psum_l.tile([128, VS], F32, tag=f"pl{ni % 2}")
                mm_group(pl[:rows, :w], ms, rows, s, w)
                emit_copy_dma(pl, 128, [(ms, rows, s, w, 0)])
            else:
                # ci == 2: ni indexes a PAIR of n-slices
                pair = N_SLICES[2 * ni : 2 * ni + 2]
                pl = psum_l.tile([128, VS], F32, tag=f"pl{ni % 2}")
                dmas = []
                for half, (s, w) in enumerate(pair):
                    mm_group(pl[64 * half : 64 * half + rows, :w], ms, rows, s, w)
                    dmas.append((ms, rows, s, w, 64 * half))
                emit_copy_dma(pl, 64 * len(pair), dmas)

        sched, post = _logits_schedule()

        # ---- phase 2: 20 serial LSTM steps (B = 16)
        cT_prev = c0T
        hT_prev = h0T
        for t in range(T):
            lo = t * BL
            pg = []
            for g in range(4):
                p = psum_g.tile([128, 4, BL], F32, tag=f"pg{g}")
                pg.append(p)
                for mi in range(4):
                    m = g * 4 + mi
                    for k in range(KT):
                        nc.tensor.matmul(
                            p[:, mi, :],
                            whhT[:, k, m * 128 : (m + 1) * 128],
                            hT_prev[:, k, :],
                            start=(k == 0),
                            stop=(k == KT - 1),
                        )
            gates = work.tile([128, GT, BL], BF16, tag="gates")
            for g in range(4):
                nc.vector.tensor_add(
                    gates[:, 4 * g : 4 * g + 4, :],
                    pg[g][:],
                    xgT[:, 4 * g : 4 * g + 4, lo : lo + BL],
                )
            act_if = work.tile([128, 8, BL], BF16, tag="actif")
            act_g = work.tile([128, 4, BL], BF16, tag="actg")
            act_o = work.tile([128, 4, BL], BF16, tag="acto")
            nc.scalar.activation(act_if[:], gates[:, 0:8, :], AF.Sigmoid)
            nc.scalar.activation(act_g[:], gates[:, 8:12, :], AF.Tanh)
            nc.scalar.activation(act_o[:], gates[:, 12:16, :], AF.Sigmoid)
            ig = work.tile([128, 4, BL], F32, tag="ig")
            fc = work.tile([128, 4, BL], F32, tag="fc")
            nc.vector.tensor_mul(ig[:], act_if[:, 0:4, :], act_g[:])
            nc.vector.tensor_mul(fc[:], act_if[:, 4:8, :], cT_prev[:])
            c_new = work.tile([128, 4, BL], F32, tag="c")
            nc.vector.tensor_add(c_new[:], fc[:], ig[:])
            tc_b = work.tile([128, 4, BL], BF16, tag="tanhc")
            nc.scalar.activation(tc_b[:], c_new[:], AF.Tanh)
            h_new = hstore[:, :, lo : lo + BL]
            nc.vector.tensor_mul(h_new, act_o[:], tc_b[:])
            cT_prev = c_new
            hT_prev = h_new
            for ci, ni in sched[t]:
                emit_logits_unit(ci, ni)

        for ci, ni in post:
            emit_logits_unit(ci, ni)

    nc.compile()
    return nc


def _prep_inputs(features, captions, h0, c0, embed_w, W_ih, W_hh, b_ih, b_hh,
                 W_lin, b_lin):
    """Host-side layout prep (data movement only). Returns per-core in_maps."""
    bf = ml_dtypes.bfloat16
    f32 = np.float32

    features = np.asarray(features, f32)
    captions = np.asarray(captions)
    h0 = np.asarray(h0, f32)
    c0 = np.asarray(c0, f32)
    embed_w = np.asarray(embed_w, f32)
    W_ih = np.asarray(W_ih, f32)
    W_hh = np.asarray(W_hh, f32)
    b_ih = np.asarray(b_ih, f32)
    b_hh = np.asarray(b_hh, f32)
    W_lin = np.asarray(W_lin, f32)
    b_lin = np.asarray(b_lin, f32)

    # xs: [B, T, E] = [features, embed(captions[:, :T-1])]
    xs = np.empty((B, T, E), f32)
    xs[:, 0, :] = features
    xs[:, 1:, :] = embed_w[captions[:, : T - 1]]

    def to_kpm(w):  # [512, M] -> [128, KT, M] with row = k*128 + p
        return np.ascontiguousarray(w.reshape(KT, 128, w.shape[1]).transpose(1, 0, 2))

    wihT = to_kpm(W_ih.T).astype(bf)
    whhT = to_kpm(W_hh.T).astype(bf)
    wlinT = to_kpm(W_lin.T).astype(bf)
    bsum = np.ascontiguousarray((b_ih + b_hh).reshape(GT, 128).T).astype(f32)
    blin = b_lin.reshape(1, V).astype(bf)

    in_maps = []
    for j in range(NC):
        sl = slice(j * BL, (j + 1) * BL)
        x = xs[sl]  # [BL, T, E]
        xsT = x.transpose(2, 1, 0).reshape(KT, 128, T * BL).transpose(1, 0, 2)
        h0T = h0[sl].T.reshape(KT, 128, BL).transpose(1, 0, 2)
        c0T = c0[sl].T.reshape(KT, 128, BL).transpose(1, 0, 2)
        in_maps.append(
            {
                "xsT": np.ascontiguousarray(xsT).astype(bf),
                "wihT": wihT,
                "whhT": whhT,
                "bsum": bsum,
                "wlinT": wlinT,
                "blin": blin,
                "h0T": np.ascontiguousarray(h0T).astype(bf),
                "c0T": np.ascontiguousarray(c0T).astype(f32),
            }
        )
    return in_maps


def kernel(**inputs) -> np.ndarray:
    maxlen = int(inputs.get("maxlen", T))
    assert maxlen == T, f"kernel hardcodes T={T}, got maxlen={maxlen}"
    use_blin = bool(np.any(np.asarray(inputs["b_lin"])))
    key = ("nc", use_blin)
    if key not in _cache:
        _cache[key] = _build_nc(use_blin)
    nc = _cache[key]
    in_maps = _prep_inputs(
        inputs["features"], inputs["captions"], inputs["h0"], inputs["c0"],
        inputs["embed_w"], inputs["W_ih"], inputs["W_hh"], inputs["b_ih"],
        inputs["b_hh"], inputs["W_lin"], inputs["b_lin"],
    )
    res = run_bass_kernel_spmd(nc, in_maps, list(range(NC)))
    # reassemble: core j rows (t*BL + b) -> full rows (t*B + j*BL + b)
    out = np.empty((T * B, V), np.float32)
    ov = out.reshape(T, NC, BL, V)
    for j in range(NC):
        ov[:, j] = res.results[j]["out"].reshape(T, BL, V)
    return out
